# revision 19
# baseline (speedup 1.0000x reference)
"""Trainium2 Bass/Tile kernel for nn_CapsuleLayer (capsule routing + FM + MF +
multi-head interacting layer).

Strategy (pure data parallel over batch, 8 cores, B_local = 128 per core):

The reference materializes inputs_hat = einsum('bfd,dnc->bnfc', x, W), a
[B,N,F,C] tensor (64 MB/core) that does not fit in SBUF.  We avoid ever
materializing it during routing by factorizing every contraction through the
small d=64 dimension:

    outputs[b,n,c] = sum_f c[b,n,f] ih[b,n,f,c] = sum_d xc[b,n,d] W[d,n,c]
        with  xc[b,n,d] = sum_f c[b,n,f] x[b,f,d]         (per-b matmul)
    delta_logits[b,n,f] = sum_c v[b,n,c] ih[b,n,f,c]
                        = sum_d x[b,f,d] ow[b,n,d]
        with  ow[b,n,d] = sum_c v[b,n,c] W[d,n,c]          (per-n matmul)

Routing runs in fp32 (the routing scores are an output; bf16 logit deltas
measured 4e-2 rel-err on hardware).  After the final softmax everything is
bf16: the streamed FM second moment S2 = sum_f (c*ih)^2 (the only place the
full inputs_hat is touched, recomputed per field f at bf16 matmul rate with
squares on the scalar engine and weighted bf16 accumulation on DVE + Pool),
and the 2-head interacting layer.  The S2/phase-2 free layout is (c, n) so the
per-(b,n) weight broadcast keeps the packed innermost stride DVE fast modes.
"""

import numpy as np
from contextlib import ExitStack

import concourse.bass as bass
import concourse.bacc as bacc
import concourse.tile as tile
from concourse import mybir
from concourse.bass_utils import run_bass_kernel_spmd
from concourse.masks import make_identity

FP = mybir.dt.float32
BF = mybir.dt.bfloat16
AX = mybir.AxisListType
ALU = mybir.AluOpType
AF = mybir.ActivationFunctionType

N_CORES = 8
B, F, D = 1024, 64, 64
N, C = 32, 64
H = 2
E = H * C  # 128
BL = B // N_CORES  # 128
ROUTINGS = 3
EPS = 1e-7


def _cp(eng_idx):
    """Alternate psum-evacuation copies between ACT (scalar) and DVE."""
    return None


def _bc(ap, dim, count):
    """Insert a broadcast (step 0) dim of size `count` at position `dim`."""
    shape = list(ap.shape)
    shape.insert(dim, count)
    return ap.unsqueeze(dim).broadcast_to(shape)


def emit_program(nc, tc, ctx, k_mf):
    # ---------------- DRAM I/O ----------------
    x_d = nc.dram_tensor("x", [BL, F, D], FP, kind="ExternalInput").ap()
    w_d = nc.dram_tensor("w", [D, N, C], FP, kind="ExternalInput").ap()
    ri_d = nc.dram_tensor("rinit", [N, F], FP, kind="ExternalInput").ap()
    kfm_d = nc.dram_tensor("kfm", [N], FP, kind="ExternalInput").ap()
    bfm_d = nc.dram_tensor("bias_fm", [C], FP, kind="ExternalInput").ap()
    bmf_d = nc.dram_tensor("bias_mf", [C], FP, kind="ExternalInput").ap()
    khi_d = nc.dram_tensor("khi", [N], FP, kind="ExternalInput").ap()
    bhi_d = nc.dram_tensor("bias_hi", [E], FP, kind="ExternalInput").ap()
    wq_d = nc.dram_tensor("wq", [C, E], FP, kind="ExternalInput").ap()
    wk_d = nc.dram_tensor("wk", [C, E], FP, kind="ExternalInput").ap()
    wv_d = nc.dram_tensor("wv", [C, E], FP, kind="ExternalInput").ap()
    wres_d = nc.dram_tensor("wres", [C, E], FP, kind="ExternalInput").ap()
    out1_d = nc.dram_tensor("out1", [BL, 4 * C], FP, kind="ExternalOutput").ap()
    rs_d = nc.dram_tensor("rs", [BL, N, F], FP, kind="ExternalOutput").ap()

    # ---------------- constant SBUF (whole kernel) ----------------
    consts = ctx.enter_context(tc.tile_pool(name="consts", bufs=1))

    ident = consts.tile([128, 128], FP)
    make_identity(nc, ident)

    w_sb = consts.tile([D, N, C], FP)  # [d, n, c]
    nc.sync.dma_start(out=w_sb, in_=w_d)
    w2h = consts.tile([D, C, N], BF)  # [d, c, n] bf16 (phase-2 moving operand)
    nc.vector.tensor_copy(out=w2h, in_=w_sb.transpose([0, 2, 1]))

    kfm_b = consts.tile([128, N], FP)
    nc.sync.dma_start(
        out=kfm_b, in_=bass.AP(tensor=kfm_d.tensor, offset=kfm_d.offset, ap=[[0, 128], [1, N]])
    )
    bfm_b = consts.tile([128, C], FP)
    nc.sync.dma_start(
        out=bfm_b, in_=bass.AP(tensor=bfm_d.tensor, offset=bfm_d.offset, ap=[[0, 128], [1, C]])
    )
    bmf_b = consts.tile([128, C], FP)
    nc.sync.dma_start(
        out=bmf_b, in_=bass.AP(tensor=bmf_d.tensor, offset=bmf_d.offset, ap=[[0, 128], [1, C]])
    )
    khi32 = consts.tile([N, 1], FP)
    nc.sync.dma_start(out=khi32, in_=khi_d.unsqueeze(1))
    khi32h = consts.tile([N, 1], BF)
    nc.scalar.copy(khi32h, khi32)
    bhi_sb = consts.tile([1, E], FP)
    nc.sync.dma_start(out=bhi_sb, in_=bhi_d.unsqueeze(0))

    wq_sb = consts.tile([C, E], FP)
    nc.sync.dma_start(out=wq_sb, in_=wq_d)
    wk_sb = consts.tile([C, E], FP)
    nc.sync.dma_start(out=wk_sb, in_=wk_d)
    wvres_sb = consts.tile([C, 2 * E], FP)
    nc.sync.dma_start(out=wvres_sb[:, 0:E], in_=wv_d)
    nc.sync.dma_start(out=wvres_sb[:, E : 2 * E], in_=wres_d)

    ri_sb = consts.tile([N, F], FP)
    nc.sync.dma_start(out=ri_sb, in_=ri_d)
    riT = consts.tile([F, N], FP)

    # ---------------- lifetime-scoped stacks ----------------
    st_xT = ExitStack()      # xT: setup .. end of phase2
    st_xF = ExitStack()      # xF/wT: setup .. end of routing
    st_bl = ExitStack()      # bl: routing
    st_life = ExitStack()    # S1/fvT2/cB: block2 .. attention
    st_rps = ExitStack()     # routing PSUM pool

    pxT = st_xT.enter_context(tc.tile_pool(name="pxT", bufs=1))
    xT = pxT.tile([D, 128, F], FP)  # [d, b, f]
    pxF = st_xF.enter_context(tc.tile_pool(name="pxF", bufs=1))
    xF = pxF.tile([F, 128, D], FP)  # [f, b, d]
    wT = pxF.tile([C, N, D], FP)  # [c, n, d]

    # ---------------- one-time transposes ----------------
    with tc.tile_pool(name="pxb", bufs=1) as pxb, tc.tile_pool(
        name="setup_ps", bufs=2, space="PSUM"
    ) as sps:
        xb = pxb.tile([128, F, D], FP)  # [b, f, d]
        nc.sync.dma_start(out=xb, in_=x_d)
        for g in range(F // 4):  # xT: transpose xb[:, f, :] -> [d, b]
            pt = sps.tile([D, 4, 128], FP, tag="t64")
            for j in range(4):
                f = 4 * g + j
                nc.tensor.transpose(pt[:, j, :], xb[:, f, :], ident)
            eng_copy = nc.scalar.copy if g % 2 else nc.vector.tensor_copy
            eng_copy(out=xT[:, :, 4 * g : 4 * g + 4], in_=pt.transpose([0, 2, 1]))
        for g in range(D // 4):  # xF: transpose xb[:, :, d] -> [f, b]
            pt = sps.tile([F, 4, 128], FP, tag="t64")
            for j in range(4):
                d = 4 * g + j
                nc.tensor.transpose(pt[:, j, :], xb[:, :, d], ident)
            eng_copy = nc.scalar.copy if g % 2 else nc.vector.tensor_copy
            eng_copy(out=xF[:, :, 4 * g : 4 * g + 4], in_=pt.transpose([0, 2, 1]))
        for g in range(N // 8):  # wT: transpose w_sb[:, n, :] ([d, c]) -> [c, d]
            pt = sps.tile([C, 8, D], FP, tag="t64")
            for j in range(8):
                n = 8 * g + j
                nc.tensor.transpose(pt[:, j, :], w_sb[:, n, :], ident[0:D, 0:D])
            nc.scalar.copy(out=wT[:, 8 * g : 8 * g + 8, :], in_=pt)
        pt = sps.tile([F, N], FP, tag="tri")
        nc.tensor.transpose(pt, ri_sb, ident[0:N, 0:N])
        nc.scalar.copy(out=riT, in_=pt)

    # ---------------- routing (fp32) ----------------
    def softmax_bl(rp, bl, c):
        # no max-subtract: |logits| stay well under exp's fp32 range
        nc.scalar.activation(c, bl, AF.Exp)
        sm = rp.tile([F, 128], FP, tag="sm")
        nc.vector.reduce_sum(out=sm, in_=c, axis=AX.X)
        rec = rp.tile([F, 128], FP, tag="rec")
        nc.vector.reciprocal(rec, sm)
        nc.gpsimd.tensor_mul(c, c, _bc(rec, 2, N))

    def xcT_pass(pp, c, xcT):
        # xcT[d, b, n] = sum_f xF[f, b, d] * c[f, b, n]   (per-b matmul)
        for g in range(8):
            pt = pp.tile([D, 16, N], FP, tag="psA")
            for j in range(16):
                b = 16 * g + j
                nc.tensor.matmul(pt[:, j, :], xF[:, b, :], c[:, b, :])
            eng_copy = nc.scalar.copy if g % 2 else nc.vector.tensor_copy
            eng_copy(out=xcT[:, 16 * g : 16 * g + 16, :], in_=pt)

    pbl = st_bl.enter_context(tc.tile_pool(name="pbl", bufs=1))
    bl = pbl.tile([F, 128, N], FP)
    nc.vector.tensor_copy(bl, _bc(riT, 1, 128))

    r_ps = st_rps.enter_context(tc.tile_pool(name="r_ps", bufs=2, space="PSUM"))

    with tc.tile_pool(name="R1", bufs=1) as rp:
        for it in range(ROUTINGS):
            c = rp.tile([F, 128, N], FP, tag="c")
            softmax_bl(rp, bl, c)
            xcT = rp.tile([D, 128, N], FP, tag="xcT")
            xcT_pass(r_ps, c, xcT)

            # o[b, n, c] stays in PSUM; squash consumes it in place
            pso = []
            for g in range(4):
                pt = r_ps.tile([128, 8, C], FP, tag="psO", bufs=4)
                for j in range(8):
                    n = 8 * g + j
                    nc.tensor.matmul(pt[:, j, :], xcT[:, :, n], w_sb[:, n, :])
                pso.append(pt)
            osq = rp.tile([128, N, C], BF, tag="osq")
            s = rp.tile([128, N], FP, tag="s")
            for g in range(4):
                nc.scalar.square(osq[:, 8 * g : 8 * g + 8, :], pso[g])
                nc.vector.reduce_sum(
                    out=s[:, 8 * g : 8 * g + 8], in_=osq[:, 8 * g : 8 * g + 8, :], axis=AX.X
                )
            nc.vector.tensor_scalar_add(s, s, EPS)
            rt = rp.tile([128, N], FP, tag="rt")
            nc.scalar.sqrt(rt, s)
            den = rp.tile([128, N], FP, tag="den")
            nc.vector.tensor_scalar_add(den, s, 0.5)
            nc.vector.reciprocal(den, den)
            scl = rp.tile([128, N], FP, tag="scl")
            nc.vector.tensor_mul(scl, rt, den)
            v = rp.tile([128, N, C], FP, tag="v")
            for g in range(4):
                nc.vector.tensor_mul(
                    v[:, 8 * g : 8 * g + 8, :], pso[g], _bc(scl[:, 8 * g : 8 * g + 8], 2, C)
                )

            for hb in range(2):  # per b-half: vT, owT, delta, bl +=
                b0 = 64 * hb
                vT = rp.tile([C, N, 64], FP, tag="vT")
                for g in range(4):
                    pt = r_ps.tile([C, 8, 64], FP, tag="psB")
                    for j in range(8):
                        n = 8 * g + j
                        nc.tensor.transpose(
                            pt[:, j, :],
                            v[b0 : b0 + 64, n, :],
                            ident[b0 : b0 + 64, b0 : b0 + 64],
                        )
                    eng_copy = nc.scalar.copy if g % 2 else nc.vector.tensor_copy
                    eng_copy(out=vT[:, 8 * g : 8 * g + 8, :], in_=pt)
                owT = rp.tile([D, N, 64], FP, tag="owT")
                for g in range(4):
                    pt = r_ps.tile([D, 8, 64], FP, tag="psB")
                    for j in range(8):
                        n = 8 * g + j
                        nc.tensor.matmul(pt[:, j, :], wT[:, n, :], vT[:, n, :])
                    eng_copy = nc.scalar.copy if g % 2 else nc.vector.tensor_copy
                    eng_copy(out=owT[:, 8 * g : 8 * g + 8, :], in_=pt)
                for g in range(4):
                    pt = r_ps.tile([F, 16, N], FP, tag="psA")
                    for j in range(16):
                        b = b0 + 16 * g + j
                        nc.tensor.matmul(pt[:, j, :], xT[:, b, :], owT[:, :, 16 * g + j])
                    dl = rp.tile([F, 16, N], FP, tag="dl", bufs=2)
                    nc.scalar.copy(out=dl, in_=pt)
                    nc.gpsimd.tensor_add(
                        bl[:, b0 + 16 * g : b0 + 16 * g + 16, :],
                        bl[:, b0 + 16 * g : b0 + 16 * g + 16, :],
                        dl,
                    )

    # ---------------- final pass (block2) ----------------
    plife = st_life.enter_context(tc.tile_pool(name="plife", bufs=1, side="right"))
    S1 = plife.tile([128, N, C], FP)   # field vectors [b, n, c]
    fvT2 = plife.tile([C, 128, N], FP)  # [c, b, n]
    cB = plife.tile([128, N, F], FP)   # final c, [b, n, f]

    with tc.tile_pool(name="B2", bufs=1) as rp:
        c = rp.tile([F, 128, N], FP, tag="c")
        softmax_bl(rp, bl, c)
        for g in range(4):
            pt = r_ps.tile([128, 8, F], FP, tag="psB")
            for j in range(8):
                n = 8 * g + j
                nc.tensor.transpose(pt[:, j, :], c[:, :, n], ident[0:F, 0:F])
            nc.scalar.copy(out=cB[:, 8 * g : 8 * g + 8, :], in_=pt)
        xcT = rp.tile([D, 128, N], FP, tag="xcT")
        xcT_pass(r_ps, c, xcT)
        for g in range(4):
            pt = r_ps.tile([128, 8, C], FP, tag="psB")
            for j in range(8):
                n = 8 * g + j
                nc.tensor.matmul(pt[:, j, :], xcT[:, :, n], w_sb[:, n, :])
            nc.scalar.copy(out=S1[:, 8 * g : 8 * g + 8, :], in_=pt)
        for hb in range(2):  # fvT2[c, b, n]
            b0 = 64 * hb
            for g in range(4):
                pt = r_ps.tile([C, 8, 64], FP, tag="psB")
                for j in range(8):
                    n = 8 * g + j
                    nc.tensor.transpose(
                        pt[:, j, :],
                        S1[b0 : b0 + 64, n, :],
                        ident[b0 : b0 + 64, b0 : b0 + 64],
                    )
                eng_copy = nc.scalar.copy if g % 2 else nc.vector.tensor_copy
                eng_copy(
                    out=fvT2[:, b0 : b0 + 64, 8 * g : 8 * g + 8],
                    in_=pt.transpose([0, 2, 1]),
                )
    st_bl.close()
    st_xF.close()
    st_rps.close()

    nc.sync.dma_start(out=rs_d, in_=cB)

    # ---------------- phase 2: S2[b, c, n] = sum_f kfm[n] c^2 ih^2 ----------------
    with tc.tile_pool(name="ph2", bufs=1) as p2, tc.tile_pool(
        name="ph2_ps", bufs=2, space="PSUM"
    ) as pp2:
        xTh = p2.tile([D, 128, F], BF)
        nc.vector.tensor_copy(xTh, xT)
        cBt = p2.tile([128, F, N], BF)  # [b, f, n]
        nc.vector.tensor_copy(out=cBt, in_=cB.transpose([0, 2, 1]))
        c2Bt = p2.tile([128, F, N], BF)  # kfm[n] * c^2, [b, f, n]
        nc.scalar.square(c2Bt, cBt)
        nc.vector.tensor_mul(c2Bt, c2Bt, _bc(kfm_b, 1, F))

        S2 = p2.tile([128, C, N], FP)  # f32 accumulator (Pool-owned)
        w2h_f = w2h.rearrange("d c n -> d (c n)")

        def ph2_f(f):
            pt = pp2.tile([128, C, N], FP, tag="ps_ih")
            ptf = pt.rearrange("b c n -> b (c n)")
            for j in range(4):
                nc.tensor.matmul(
                    ptf[:, 512 * j : 512 * (j + 1)],
                    xTh[:, :, f],
                    w2h_f[:, 512 * j : 512 * (j + 1)],
                )
            sq = p2.tile([128, C, N], BF, tag="sq", bufs=2, name=f"sq{f}")
            nc.scalar.square(sq, pt)
            tmp = p2.tile([128, C, N], BF, tag=f"tmp{f % 4}", bufs=2, name=f"tmp{f}")
            nc.vector.tensor_mul(tmp, sq, _bc(c2Bt[:, f, :], 1, C))
            return tmp

        for fq in range(F // 4):
            tms = [ph2_f(4 * fq + i) for i in range(4)]
            t12 = p2.tile([128, C, N], BF, tag="t12", bufs=2, name=f"t12_{fq}")
            nc.vector.tensor_add(t12, tms[0], tms[1])
            t34 = p2.tile([128, C, N], BF, tag="t34", bufs=2, name=f"t34_{fq}")
            nc.vector.tensor_add(t34, tms[2], tms[3])
            t1234 = p2.tile([128, C, N], BF, tag="t1234", bufs=2, name=f"t1234_{fq}")
            nc.vector.tensor_add(t1234, t12, t34)
            if fq == 0:
                nc.gpsimd.tensor_copy(out=S2, in_=t1234)
            else:
                nc.gpsimd.tensor_add(S2, S2, t1234)

        # ---------------- FM + MF tails ----------------
        with tc.tile_pool(name="tails", bufs=1) as tp:
            out1_t = tp.tile([128, 2 * C], FP)
            s1sq = tp.tile([128, N, C], FP)
            nc.scalar.square(s1sq, S1)
            t0 = tp.tile([128, N, C], FP)
            nc.vector.tensor_mul(t0, s1sq, _bc(kfm_b, 2, C))
            hA = tp.tile([128, C], FP)
            nc.vector.reduce_sum(out=hA, in_=t0.transpose([0, 2, 1]), axis=AX.X)
            hB = tp.tile([128, C], FP)
            nc.vector.reduce_sum(out=hB, in_=S2, axis=AX.X)
            d1 = tp.tile([128, C], FP)
            nc.vector.tensor_sub(d1, hA, hB)
            nc.vector.tensor_add(out1_t[:, 0:C], d1, bfm_b)

            ssum = tp.tile([128, C], FP)
            nc.vector.reduce_sum(out=ssum, in_=S1.transpose([0, 2, 1]), axis=AX.X)
            sqsum = tp.tile([128, C], FP)
            nc.vector.reduce_sum(out=sqsum, in_=s1sq.transpose([0, 2, 1]), axis=AX.X)
            ss2 = tp.tile([128, C], FP)
            nc.scalar.square(ss2, ssum)
            dd = tp.tile([128, C], FP)
            nc.vector.tensor_sub(dd, ss2, sqsum)
            nc.vector.scalar_tensor_tensor(
                out=out1_t[:, C : 2 * C],
                in0=dd,
                scalar=0.5 * k_mf,
                in1=bmf_b,
                op0=ALU.mult,
                op1=ALU.add,
            )
            nc.sync.dma_start(out=out1_d[:, 0 : 2 * C], in_=out1_t)
    st_xT.close()

    # ---------------- interacting layer (2-head attention), bf16 ----------------
    # All matmul outputs must sit at PSUM partition base 0 (walrus), so the
    # attention middle runs in [n|m, ...] 32-partition layouts, b-quarters.
    with tc.tile_pool(name="attn", bufs=1) as ap_, tc.tile_pool(
        name="attn_ps", bufs=2, space="PSUM"
    ) as app:
        fvT2h = ap_.tile([C, 128, N], BF)
        nc.scalar.copy(fvT2h, fvT2)
        wqh = ap_.tile([C, E], BF)
        nc.scalar.copy(wqh, wq_sb)
        wkh = ap_.tile([C, E], BF)
        nc.scalar.copy(wkh, wk_sb)
        wvresh = ap_.tile([C, 2 * E], BF)
        nc.scalar.copy(wvresh, wvres_sb)

        fvT2h_f = fvT2h.rearrange("c b n -> c (b n)")
        # q/k projections: [e, b, n]
        qTh = ap_.tile([E, 128, N], BF)
        kTh = ap_.tile([E, 128, N], BF)
        for (wh, dst) in ((wqh, qTh), (wkh, kTh)):
            dflat = dst.rearrange("e b n -> e (b n)")
            for j in range(8):
                pt = app.tile([E, 512], FP, tag="ps_q")
                nc.tensor.matmul(pt, wh, fvT2h_f[:, 512 * j : 512 * (j + 1)])
                eng_copy = nc.scalar.copy if j % 2 else nc.vector.tensor_copy
                eng_copy(out=dflat[:, 512 * j : 512 * (j + 1)], in_=pt)

        QB = 32  # b-quarter
        for qb in range(4):
            b0 = QB * qb
            # v|res projection for this quarter: out [n, 2E] per b
            v8 = ap_.tile([N, QB, E], BF, tag="v8")
            res8 = ap_.tile([N, QB, E], BF, tag="res8")
            for g in range(4):
                pt = app.tile([N, 8, 2 * E], FP, tag="ps_q")
                for j in range(8):
                    b = b0 + 8 * g + j
                    nc.tensor.matmul(pt[:, j, :], fvT2h[:, b, :], wvresh)
                nc.vector.tensor_copy(out=v8[:, 8 * g : 8 * g + 8, :], in_=pt[:, :, 0:E])
                nc.scalar.copy(out=res8[:, 8 * g : 8 * g + 8, :], in_=pt[:, :, E : 2 * E])

            # scores [n, h, b, m]
            sc = ap_.tile([N, H, QB, N], FP, tag="sc")
            pt = app.tile([N, H, QB, N], FP, tag="ps_q")
            for j in range(QB):
                b = b0 + j
                for h in range(H):
                    nc.tensor.matmul(
                        pt[:, h, j, :],
                        qTh[64 * h : 64 * h + 64, b, :],
                        kTh[64 * h : 64 * h + 64, b, :],
                    )
            nc.scalar.copy(out=sc, in_=pt)

            # softmax over m (innermost), in place
            mxa = ap_.tile([N, H, QB], FP, tag="mxa")
            nc.vector.reduce_max(out=mxa, in_=sc, axis=AX.X)
            nc.vector.tensor_sub(sc, sc, _bc(mxa, 3, N))
            nc.scalar.activation(sc, sc, AF.Exp)
            sma = ap_.tile([N, H, QB], FP, tag="sma")
            nc.vector.reduce_sum(out=sma, in_=sc, axis=AX.X)
            nc.vector.reciprocal(sma, sma)
            nc.gpsimd.tensor_mul(sc, sc, _bc(sma, 3, N))

            # transpose each attention tile -> ATh (bf16)
            ATh = ap_.tile([N, H, QB, N], BF, tag="ATh")
            pt = app.tile([N, H, QB, N], FP, tag="ps_q")
            for j in range(QB):
                for h in range(H):
                    nc.tensor.transpose(pt[:, h, j, :], sc[:, h, j, :], ident[0:N, 0:N])
            nc.vector.tensor_copy(out=ATh, in_=pt)

            # out_attn + res -> relu -> fv2 [n, b, (h e')]
            fv2 = ap_.tile([N, QB, E], BF, tag="fv2")
            for half in range(2):
                pt = app.tile([N, 16, H, C], FP, tag="ps_q")
                for j in range(16 * half, 16 * half + 16):
                    for h in range(H):
                        nc.tensor.matmul(
                            pt[:, j % 16, h, :],
                            ATh[:, h, j, :],
                            v8[:, j, 64 * h : 64 * h + 64],
                        )
                fvs = fv2[:, 16 * half : 16 * half + 16, :]
                nc.vector.tensor_add(
                    fvs, pt.rearrange("n j h c -> n j (h c)"),
                    res8[:, 16 * half : 16 * half + 16, :],
                )
                nc.vector.tensor_scalar_max(fvs, fvs, 0.0)

            # high_int quarter: sum_n khi[n] * fv2 + bias_hi
            hi_sb = ap_.tile([1, QB, E], FP, tag="hi_sb")
            fv2f = fv2.rearrange("n b e -> n (b e)")
            for hf in range(2):
                pt = app.tile([1, 16, E], FP, tag="ps_q")
                ptf = pt.rearrange("o b e -> o (b e)")
                for j in range(4):
                    nc.tensor.matmul(
                        ptf[:, 512 * j : 512 * (j + 1)],
                        khi32h,
                        fv2f[:, 2048 * hf + 512 * j : 2048 * hf + 512 * (j + 1)],
                    )
                nc.vector.scalar_tensor_tensor(
                    out=hi_sb[:, 16 * hf : 16 * (hf + 1), :],
                    in0=pt,
                    scalar=1.0,
                    in1=_bc(bhi_sb, 1, 16),
                    op0=ALU.mult,
                    op1=ALU.add,
                )
            nc.sync.dma_start(out=out1_d[b0 : b0 + QB, 2 * C : 4 * C], in_=hi_sb)
    st_life.close()


_PROG_CACHE = {}


def _get_program(k_mf):
    key = float(k_mf)
    if key not in _PROG_CACHE:
        nc = bacc.Bacc("TRN2", target_bir_lowering=False, debug=False)
        with tile.TileContext(nc) as tc:
            with ExitStack() as ctx:
                emit_program(nc, tc, ctx, key)
        nc.compile()
        _PROG_CACHE[key] = nc
    return _PROG_CACHE[key]


def _prep_in_maps(inputs):
    f32 = lambda a: np.ascontiguousarray(np.asarray(a), dtype=np.float32)
    x = f32(inputs["x"])
    shared = {
        "w": f32(inputs["reweight_W"]),
        "rinit": f32(inputs["routing_init"]),
        "kfm": f32(inputs["kernel_fm"]).reshape(N),
        "bias_fm": f32(inputs["bias_fm"]).reshape(C),
        "bias_mf": f32(inputs["bias_mf"]).reshape(C),
        "khi": f32(inputs["kernel_highint"]).reshape(N),
        "bias_hi": f32(inputs["bias_highint"]).reshape(E),
        "wq": f32(inputs["W_query"]),
        "wk": f32(inputs["W_key"]),
        "wv": f32(inputs["W_value"]),
        "wres": f32(inputs["W_res"]),
    }
    in_maps = []
    for i in range(N_CORES):
        m = dict(shared)
        m["x"] = np.ascontiguousarray(x[i * BL : (i + 1) * BL])
        in_maps.append(m)
    k_mf = float(np.asarray(inputs["kernel_mf"]).reshape(-1)[0])
    return in_maps, k_mf


def kernel(**inputs):
    in_maps, k_mf = _prep_in_maps(inputs)
    nc = _get_program(k_mf)
    res = run_bass_kernel_spmd(nc, in_maps, list(range(N_CORES)))
    out1 = np.concatenate([res.results[i]["out1"] for i in range(N_CORES)], axis=0)
    rs = np.concatenate([res.results[i]["rs"] for i in range(N_CORES)], axis=0)
    return out1, rs.reshape(B, N, F, 1)


# revision 20
# speedup vs baseline: 1.0430x; 1.0430x over previous
"""Trainium2 Bass/Tile kernel for nn_CapsuleLayer (capsule routing + FM + MF +
multi-head interacting layer).

Strategy (pure data parallel over batch, 8 cores, B_local = 128 per core):

The reference materializes inputs_hat = einsum('bfd,dnc->bnfc', x, W), a
[B,N,F,C] tensor (64 MB/core) that does not fit in SBUF.  We avoid ever
materializing it during routing by factorizing every contraction through the
small d=64 dimension:

    outputs[b,n,c] = sum_f c[b,n,f] ih[b,n,f,c] = sum_d xc[b,n,d] W[d,n,c]
        with  xc[b,n,d] = sum_f c[b,n,f] x[b,f,d]         (per-b matmul)
    delta_logits[b,n,f] = sum_c v[b,n,c] ih[b,n,f,c]
                        = sum_d x[b,f,d] ow[b,n,d]
        with  ow[b,n,d] = sum_c v[b,n,c] W[d,n,c]          (per-n matmul)

Routing runs in fp32 (the routing scores are an output; bf16 logit deltas
measured 4e-2 rel-err on hardware).  After the final softmax everything is
bf16: the streamed FM second moment S2 = sum_f (c*ih)^2 (the only place the
full inputs_hat is touched, recomputed per field f at bf16 matmul rate with
squares on the scalar engine and weighted bf16 accumulation on DVE + Pool),
and the 2-head interacting layer.  The S2/phase-2 free layout is (c, n) so the
per-(b,n) weight broadcast keeps the packed innermost stride DVE fast modes.
"""

import numpy as np
from contextlib import ExitStack

import concourse.bass as bass
import concourse.bacc as bacc
import concourse.tile as tile
from concourse import mybir
from concourse.bass_utils import run_bass_kernel_spmd
from concourse.masks import make_identity

FP = mybir.dt.float32
BF = mybir.dt.bfloat16
AX = mybir.AxisListType
ALU = mybir.AluOpType
AF = mybir.ActivationFunctionType

N_CORES = 8
B, F, D = 1024, 64, 64
N, C = 32, 64
H = 2
E = H * C  # 128
BL = B // N_CORES  # 128
ROUTINGS = 3
EPS = 1e-7


def _cp(eng_idx):
    """Alternate psum-evacuation copies between ACT (scalar) and DVE."""
    return None


def _bc(ap, dim, count):
    """Insert a broadcast (step 0) dim of size `count` at position `dim`."""
    shape = list(ap.shape)
    shape.insert(dim, count)
    return ap.unsqueeze(dim).broadcast_to(shape)


def emit_program(nc, tc, ctx, k_mf):
    # ---------------- DRAM I/O ----------------
    x_d = nc.dram_tensor("x", [BL, F, D], FP, kind="ExternalInput").ap()
    w_d = nc.dram_tensor("w", [D, N, C], FP, kind="ExternalInput").ap()
    ri_d = nc.dram_tensor("rinit", [N, F], FP, kind="ExternalInput").ap()
    kfm_d = nc.dram_tensor("kfm", [N], FP, kind="ExternalInput").ap()
    bfm_d = nc.dram_tensor("bias_fm", [C], FP, kind="ExternalInput").ap()
    bmf_d = nc.dram_tensor("bias_mf", [C], FP, kind="ExternalInput").ap()
    khi_d = nc.dram_tensor("khi", [N], FP, kind="ExternalInput").ap()
    bhi_d = nc.dram_tensor("bias_hi", [E], FP, kind="ExternalInput").ap()
    wq_d = nc.dram_tensor("wq", [C, E], FP, kind="ExternalInput").ap()
    wk_d = nc.dram_tensor("wk", [C, E], FP, kind="ExternalInput").ap()
    wv_d = nc.dram_tensor("wv", [C, E], FP, kind="ExternalInput").ap()
    wres_d = nc.dram_tensor("wres", [C, E], FP, kind="ExternalInput").ap()
    out1_d = nc.dram_tensor("out1", [BL, 4 * C], FP, kind="ExternalOutput").ap()
    rs_d = nc.dram_tensor("rs", [BL, N, F], FP, kind="ExternalOutput").ap()

    # ---------------- constant SBUF (whole kernel) ----------------
    consts = ctx.enter_context(tc.tile_pool(name="consts", bufs=1))

    ident = consts.tile([128, 128], FP)
    make_identity(nc, ident)

    w_sb = consts.tile([D, N, C], FP)  # [d, n, c]
    nc.sync.dma_start(out=w_sb, in_=w_d)
    w2h = consts.tile([D, C, N], BF)  # [d, c, n] bf16 (phase-2 moving operand)
    nc.vector.tensor_copy(out=w2h, in_=w_sb.transpose([0, 2, 1]))

    kfm_b = consts.tile([128, N], FP)
    nc.sync.dma_start(
        out=kfm_b, in_=bass.AP(tensor=kfm_d.tensor, offset=kfm_d.offset, ap=[[0, 128], [1, N]])
    )
    bfm_b = consts.tile([128, C], FP)
    nc.sync.dma_start(
        out=bfm_b, in_=bass.AP(tensor=bfm_d.tensor, offset=bfm_d.offset, ap=[[0, 128], [1, C]])
    )
    bmf_b = consts.tile([128, C], FP)
    nc.sync.dma_start(
        out=bmf_b, in_=bass.AP(tensor=bmf_d.tensor, offset=bmf_d.offset, ap=[[0, 128], [1, C]])
    )
    khi32 = consts.tile([N, 1], FP)
    nc.sync.dma_start(out=khi32, in_=khi_d.unsqueeze(1))
    khi32h = consts.tile([N, 1], BF)
    nc.scalar.copy(khi32h, khi32)
    bhi_sb = consts.tile([1, E], FP)
    nc.sync.dma_start(out=bhi_sb, in_=bhi_d.unsqueeze(0))

    wq_sb = consts.tile([C, E], FP)
    nc.sync.dma_start(out=wq_sb, in_=wq_d)
    wk_sb = consts.tile([C, E], FP)
    nc.sync.dma_start(out=wk_sb, in_=wk_d)
    wvres_sb = consts.tile([C, 2 * E], FP)
    nc.sync.dma_start(out=wvres_sb[:, 0:E], in_=wv_d)
    nc.sync.dma_start(out=wvres_sb[:, E : 2 * E], in_=wres_d)

    ri_sb = consts.tile([N, F], FP)
    nc.sync.dma_start(out=ri_sb, in_=ri_d)
    riT = consts.tile([F, N], FP)

    # ---------------- lifetime-scoped stacks ----------------
    st_xT = ExitStack()      # xT: setup .. end of phase2
    st_xF = ExitStack()      # xF/wT: setup .. end of routing
    st_bl = ExitStack()      # bl: routing
    st_life = ExitStack()    # S1/fvT2/cB: block2 .. attention
    st_rps = ExitStack()     # routing PSUM pool

    pxT = st_xT.enter_context(tc.tile_pool(name="pxT", bufs=1))
    xT = pxT.tile([D, 128, F], FP)  # [d, b, f]
    pxF = st_xF.enter_context(tc.tile_pool(name="pxF", bufs=1))
    xF = pxF.tile([F, 128, D], FP)  # [f, b, d]
    wT = pxF.tile([C, N, D], FP)  # [c, n, d]

    # ---------------- one-time transposes ----------------
    with tc.tile_pool(name="pxb", bufs=1) as pxb, tc.tile_pool(
        name="setup_ps", bufs=2, space="PSUM"
    ) as sps:
        xb = pxb.tile([128, F, D], FP)  # [b, f, d]
        nc.sync.dma_start(out=xb, in_=x_d)
        for g in range(F // 4):  # xT: transpose xb[:, f, :] -> [d, b]
            pt = sps.tile([D, 4, 128], FP, tag="t64")
            for j in range(4):
                f = 4 * g + j
                nc.tensor.transpose(pt[:, j, :], xb[:, f, :], ident)
            eng_copy = nc.scalar.copy if g % 2 else nc.vector.tensor_copy
            eng_copy(out=xT[:, :, 4 * g : 4 * g + 4], in_=pt.transpose([0, 2, 1]))
        for g in range(D // 4):  # xF: transpose xb[:, :, d] -> [f, b]
            pt = sps.tile([F, 4, 128], FP, tag="t64")
            for j in range(4):
                d = 4 * g + j
                nc.tensor.transpose(pt[:, j, :], xb[:, :, d], ident)
            eng_copy = nc.scalar.copy if g % 2 else nc.vector.tensor_copy
            eng_copy(out=xF[:, :, 4 * g : 4 * g + 4], in_=pt.transpose([0, 2, 1]))
        for g in range(N // 8):  # wT: transpose w_sb[:, n, :] ([d, c]) -> [c, d]
            pt = sps.tile([C, 8, D], FP, tag="t64")
            for j in range(8):
                n = 8 * g + j
                nc.tensor.transpose(pt[:, j, :], w_sb[:, n, :], ident[0:D, 0:D])
            nc.scalar.copy(out=wT[:, 8 * g : 8 * g + 8, :], in_=pt)
        pt = sps.tile([F, N], FP, tag="tri")
        nc.tensor.transpose(pt, ri_sb, ident[0:N, 0:N])
        nc.scalar.copy(out=riT, in_=pt)

    # ---------------- routing (fp32) ----------------
    def softmax_bl(rp, bl, c):
        # no max-subtract: |logits| stay well under exp's fp32 range.
        # Split by b-halves so downstream per-b matmuls overlap the 2nd half.
        for hh in range(2):
            b0 = 64 * hh
            cs = c[:, b0 : b0 + 64, :]
            nc.scalar.activation(cs, bl[:, b0 : b0 + 64, :], AF.Exp)
            sm = rp.tile([F, 64], FP, tag="sm", bufs=2, name=f"sm{hh}")
            nc.vector.reduce_sum(out=sm, in_=cs, axis=AX.X)
            rec = rp.tile([F, 64], FP, tag="rec", bufs=2, name=f"rec{hh}")
            nc.vector.reciprocal(rec, sm)
            nc.gpsimd.tensor_mul(cs, cs, _bc(rec, 2, N))

    def xcT_pass(pp, c, xcT):
        # xcT[d, b, n] = sum_f xF[f, b, d] * c[f, b, n]   (per-b matmul)
        for g in range(8):
            pt = pp.tile([D, 16, N], FP, tag="psA")
            for j in range(16):
                b = 16 * g + j
                nc.tensor.matmul(pt[:, j, :], xF[:, b, :], c[:, b, :])
            eng_copy = nc.scalar.copy if g % 2 else nc.vector.tensor_copy
            eng_copy(out=xcT[:, 16 * g : 16 * g + 16, :], in_=pt)

    pbl = st_bl.enter_context(tc.tile_pool(name="pbl", bufs=1))
    bl = pbl.tile([F, 128, N], FP)
    nc.vector.tensor_copy(bl, _bc(riT, 1, 128))

    r_ps = st_rps.enter_context(tc.tile_pool(name="r_ps", bufs=2, space="PSUM"))

    with tc.tile_pool(name="R1", bufs=1) as rp:
        for it in range(ROUTINGS):
            c = rp.tile([F, 128, N], FP, tag="c")
            softmax_bl(rp, bl, c)
            xcT = rp.tile([D, 128, N], FP, tag="xcT")
            xcT_pass(r_ps, c, xcT)

            # o[b, n, c] stays in PSUM; squash consumes it in place
            pso = []
            for g in range(4):
                pt = r_ps.tile([128, 8, C], FP, tag="psO", bufs=4)
                for j in range(8):
                    n = 8 * g + j
                    nc.tensor.matmul(pt[:, j, :], xcT[:, :, n], w_sb[:, n, :])
                pso.append(pt)
            osq = rp.tile([128, N, C], BF, tag="osq")
            s = rp.tile([128, N], FP, tag="s")
            for g in range(4):
                nc.scalar.square(osq[:, 8 * g : 8 * g + 8, :], pso[g])
                nc.vector.reduce_sum(
                    out=s[:, 8 * g : 8 * g + 8], in_=osq[:, 8 * g : 8 * g + 8, :], axis=AX.X
                )
            nc.vector.tensor_scalar_add(s, s, EPS)
            rt = rp.tile([128, N], FP, tag="rt")
            nc.scalar.sqrt(rt, s)
            den = rp.tile([128, N], FP, tag="den")
            nc.vector.tensor_scalar_add(den, s, 0.5)
            nc.vector.reciprocal(den, den)
            scl = rp.tile([128, N], FP, tag="scl")
            nc.vector.tensor_mul(scl, rt, den)
            v = rp.tile([128, N, C], FP, tag="v")
            for g in range(4):
                nc.vector.tensor_mul(
                    v[:, 8 * g : 8 * g + 8, :], pso[g], _bc(scl[:, 8 * g : 8 * g + 8], 2, C)
                )

            for hb in range(2):  # per b-half: vT, owT, delta, bl +=
                b0 = 64 * hb
                vT = rp.tile([C, N, 64], FP, tag="vT")
                for g in range(4):
                    pt = r_ps.tile([C, 8, 64], FP, tag="psB")
                    for j in range(8):
                        n = 8 * g + j
                        nc.tensor.transpose(
                            pt[:, j, :],
                            v[b0 : b0 + 64, n, :],
                            ident[b0 : b0 + 64, b0 : b0 + 64],
                        )
                    eng_copy = nc.scalar.copy if g % 2 else nc.vector.tensor_copy
                    eng_copy(out=vT[:, 8 * g : 8 * g + 8, :], in_=pt)
                owT = rp.tile([D, N, 64], FP, tag="owT")
                for g in range(4):
                    pt = r_ps.tile([D, 8, 64], FP, tag="psB")
                    for j in range(8):
                        n = 8 * g + j
                        nc.tensor.matmul(pt[:, j, :], wT[:, n, :], vT[:, n, :])
                    eng_copy = nc.scalar.copy if g % 2 else nc.vector.tensor_copy
                    eng_copy(out=owT[:, 8 * g : 8 * g + 8, :], in_=pt)
                for g in range(4):
                    pt = r_ps.tile([F, 16, N], FP, tag="psA")
                    for j in range(16):
                        b = b0 + 16 * g + j
                        nc.tensor.matmul(pt[:, j, :], xT[:, b, :], owT[:, :, 16 * g + j])
                    dl = rp.tile([F, 16, N], FP, tag="dl", bufs=2)
                    nc.scalar.copy(out=dl, in_=pt)
                    nc.gpsimd.tensor_add(
                        bl[:, b0 + 16 * g : b0 + 16 * g + 16, :],
                        bl[:, b0 + 16 * g : b0 + 16 * g + 16, :],
                        dl,
                    )

    # ---------------- final pass (block2) ----------------
    plife = st_life.enter_context(tc.tile_pool(name="plife", bufs=1, side="right"))
    S1 = plife.tile([128, N, C], FP)   # field vectors [b, n, c]
    fvT2 = plife.tile([C, 128, N], FP)  # [c, b, n]
    cB = plife.tile([128, N, F], FP)   # final c, [b, n, f]

    with tc.tile_pool(name="B2", bufs=1) as rp:
        c = rp.tile([F, 128, N], FP, tag="c")
        softmax_bl(rp, bl, c)
        for g in range(4):
            pt = r_ps.tile([128, 8, F], FP, tag="psB")
            for j in range(8):
                n = 8 * g + j
                nc.tensor.transpose(pt[:, j, :], c[:, :, n], ident[0:F, 0:F])
            nc.scalar.copy(out=cB[:, 8 * g : 8 * g + 8, :], in_=pt)
        xcT = rp.tile([D, 128, N], FP, tag="xcT")
        xcT_pass(r_ps, c, xcT)
        for g in range(4):
            pt = r_ps.tile([128, 8, C], FP, tag="psB")
            for j in range(8):
                n = 8 * g + j
                nc.tensor.matmul(pt[:, j, :], xcT[:, :, n], w_sb[:, n, :])
            nc.scalar.copy(out=S1[:, 8 * g : 8 * g + 8, :], in_=pt)
        for hb in range(2):  # fvT2[c, b, n]
            b0 = 64 * hb
            for g in range(4):
                pt = r_ps.tile([C, 8, 64], FP, tag="psB")
                for j in range(8):
                    n = 8 * g + j
                    nc.tensor.transpose(
                        pt[:, j, :],
                        S1[b0 : b0 + 64, n, :],
                        ident[b0 : b0 + 64, b0 : b0 + 64],
                    )
                eng_copy = nc.scalar.copy if g % 2 else nc.vector.tensor_copy
                eng_copy(
                    out=fvT2[:, b0 : b0 + 64, 8 * g : 8 * g + 8],
                    in_=pt.transpose([0, 2, 1]),
                )
    st_bl.close()
    st_xF.close()
    st_rps.close()

    nc.sync.dma_start(out=rs_d, in_=cB)

    # ---------------- phase 2: S2[b, c, n] = sum_f kfm[n] c^2 ih^2 ----------------
    with tc.tile_pool(name="ph2", bufs=1) as p2, tc.tile_pool(
        name="ph2_ps", bufs=2, space="PSUM"
    ) as pp2:
        xTh = p2.tile([D, 128, F], BF)
        nc.vector.tensor_copy(xTh, xT)
        cBt = p2.tile([128, F, N], BF)  # [b, f, n]
        nc.vector.tensor_copy(out=cBt, in_=cB.transpose([0, 2, 1]))
        c2Bt = p2.tile([128, F, N], BF)  # kfm[n] * c^2, [b, f, n]
        nc.scalar.square(c2Bt, cBt)
        nc.vector.tensor_mul(c2Bt, c2Bt, _bc(kfm_b, 1, F))

        S2 = p2.tile([128, C, N], FP)  # f32 accumulator (Pool-owned)
        w2h_f = w2h.rearrange("d c n -> d (c n)")

        def ph2_f(f):
            pt = pp2.tile([128, C, N], FP, tag="ps_ih")
            ptf = pt.rearrange("b c n -> b (c n)")
            for j in range(4):
                nc.tensor.matmul(
                    ptf[:, 512 * j : 512 * (j + 1)],
                    xTh[:, :, f],
                    w2h_f[:, 512 * j : 512 * (j + 1)],
                )
            sq = p2.tile([128, C, N], BF, tag="sq", bufs=2, name=f"sq{f}")
            nc.scalar.square(sq, pt)
            tmp = p2.tile([128, C, N], BF, tag=f"tmp{f % 4}", bufs=2, name=f"tmp{f}")
            nc.vector.tensor_mul(tmp, sq, _bc(c2Bt[:, f, :], 1, C))
            return tmp

        for fq in range(F // 4):
            tms = [ph2_f(4 * fq + i) for i in range(4)]
            t12 = p2.tile([128, C, N], BF, tag="t12", bufs=2, name=f"t12_{fq}")
            nc.vector.tensor_add(t12, tms[0], tms[1])
            t34 = p2.tile([128, C, N], BF, tag="t34", bufs=2, name=f"t34_{fq}")
            nc.vector.tensor_add(t34, tms[2], tms[3])
            t1234 = p2.tile([128, C, N], BF, tag="t1234", bufs=2, name=f"t1234_{fq}")
            nc.vector.tensor_add(t1234, t12, t34)
            if fq == 0:
                nc.gpsimd.tensor_copy(out=S2, in_=t1234)
            else:
                nc.gpsimd.tensor_add(S2, S2, t1234)

        # ---------------- FM + MF tails ----------------
        with tc.tile_pool(name="tails", bufs=1) as tp:
            out1_t = tp.tile([128, 2 * C], FP)
            s1sq = tp.tile([128, N, C], FP)
            nc.scalar.square(s1sq, S1)
            t0 = tp.tile([128, N, C], FP)
            nc.vector.tensor_mul(t0, s1sq, _bc(kfm_b, 2, C))
            hA = tp.tile([128, C], FP)
            nc.vector.reduce_sum(out=hA, in_=t0.transpose([0, 2, 1]), axis=AX.X)
            hB = tp.tile([128, C], FP)
            nc.vector.reduce_sum(out=hB, in_=S2, axis=AX.X)
            d1 = tp.tile([128, C], FP)
            nc.vector.tensor_sub(d1, hA, hB)
            nc.vector.tensor_add(out1_t[:, 0:C], d1, bfm_b)

            ssum = tp.tile([128, C], FP)
            nc.vector.reduce_sum(out=ssum, in_=S1.transpose([0, 2, 1]), axis=AX.X)
            sqsum = tp.tile([128, C], FP)
            nc.vector.reduce_sum(out=sqsum, in_=s1sq.transpose([0, 2, 1]), axis=AX.X)
            ss2 = tp.tile([128, C], FP)
            nc.scalar.square(ss2, ssum)
            dd = tp.tile([128, C], FP)
            nc.vector.tensor_sub(dd, ss2, sqsum)
            nc.vector.scalar_tensor_tensor(
                out=out1_t[:, C : 2 * C],
                in0=dd,
                scalar=0.5 * k_mf,
                in1=bmf_b,
                op0=ALU.mult,
                op1=ALU.add,
            )
            nc.sync.dma_start(out=out1_d[:, 0 : 2 * C], in_=out1_t)
    st_xT.close()

    # ---------------- interacting layer (2-head attention), bf16 ----------------
    # All matmul outputs must sit at PSUM partition base 0 (walrus), so the
    # attention middle runs in [n|m, ...] 32-partition layouts, b-quarters.
    with tc.tile_pool(name="attn", bufs=1) as ap_, tc.tile_pool(
        name="attn_ps", bufs=2, space="PSUM"
    ) as app:
        fvT2h = ap_.tile([C, 128, N], BF)
        nc.scalar.copy(fvT2h, fvT2)
        wqh = ap_.tile([C, E], BF)
        nc.scalar.copy(wqh, wq_sb)
        wkh = ap_.tile([C, E], BF)
        nc.scalar.copy(wkh, wk_sb)
        wvresh = ap_.tile([C, 2 * E], BF)
        nc.scalar.copy(wvresh, wvres_sb)

        fvT2h_f = fvT2h.rearrange("c b n -> c (b n)")
        # q/k projections: [e, b, n]
        qTh = ap_.tile([E, 128, N], BF)
        kTh = ap_.tile([E, 128, N], BF)
        for (wh, dst) in ((wqh, qTh), (wkh, kTh)):
            dflat = dst.rearrange("e b n -> e (b n)")
            for j in range(8):
                pt = app.tile([E, 512], FP, tag="ps_q")
                nc.tensor.matmul(pt, wh, fvT2h_f[:, 512 * j : 512 * (j + 1)])
                eng_copy = nc.scalar.copy if j % 2 else nc.vector.tensor_copy
                eng_copy(out=dflat[:, 512 * j : 512 * (j + 1)], in_=pt)

        QB = 32  # b-quarter
        for qb in range(4):
            b0 = QB * qb
            # v|res projection for this quarter: out [n, 2E] per b
            v8 = ap_.tile([N, QB, E], BF, tag="v8", bufs=2)
            res8 = ap_.tile([N, QB, E], BF, tag="res8", bufs=2)
            for g in range(4):
                pt = app.tile([N, 8, 2 * E], FP, tag="ps_q")
                for j in range(8):
                    b = b0 + 8 * g + j
                    nc.tensor.matmul(pt[:, j, :], fvT2h[:, b, :], wvresh)
                nc.vector.tensor_copy(out=v8[:, 8 * g : 8 * g + 8, :], in_=pt[:, :, 0:E])
                nc.scalar.copy(out=res8[:, 8 * g : 8 * g + 8, :], in_=pt[:, :, E : 2 * E])

            # scores [n, h, b, m]
            sc = ap_.tile([N, H, QB, N], FP, tag="sc", bufs=2)
            pt = app.tile([N, H, QB, N], FP, tag="ps_q")
            for j in range(QB):
                b = b0 + j
                for h in range(H):
                    nc.tensor.matmul(
                        pt[:, h, j, :],
                        qTh[64 * h : 64 * h + 64, b, :],
                        kTh[64 * h : 64 * h + 64, b, :],
                    )
            nc.scalar.copy(out=sc, in_=pt)

            # softmax over m (innermost), in place
            mxa = ap_.tile([N, H, QB], FP, tag="mxa", bufs=2)
            nc.vector.reduce_max(out=mxa, in_=sc, axis=AX.X)
            nc.vector.tensor_sub(sc, sc, _bc(mxa, 3, N))
            nc.scalar.activation(sc, sc, AF.Exp)
            sma = ap_.tile([N, H, QB], FP, tag="sma", bufs=2)
            nc.vector.reduce_sum(out=sma, in_=sc, axis=AX.X)
            nc.vector.reciprocal(sma, sma)
            nc.gpsimd.tensor_mul(sc, sc, _bc(sma, 3, N))

            # transpose each attention tile -> ATh (bf16)
            ATh = ap_.tile([N, H, QB, N], BF, tag="ATh", bufs=2)
            pt = app.tile([N, H, QB, N], FP, tag="ps_q")
            for j in range(QB):
                for h in range(H):
                    nc.tensor.transpose(pt[:, h, j, :], sc[:, h, j, :], ident[0:N, 0:N])
            nc.vector.tensor_copy(out=ATh, in_=pt)

            # out_attn + res -> relu -> fv2 [n, b, (h e')]
            fv2 = ap_.tile([N, QB, E], BF, tag="fv2", bufs=2)
            for half in range(2):
                pt = app.tile([N, 16, H, C], FP, tag="ps_q")
                for j in range(16 * half, 16 * half + 16):
                    for h in range(H):
                        nc.tensor.matmul(
                            pt[:, j % 16, h, :],
                            ATh[:, h, j, :],
                            v8[:, j, 64 * h : 64 * h + 64],
                        )
                fvs = fv2[:, 16 * half : 16 * half + 16, :]
                nc.vector.tensor_add(
                    fvs, pt.rearrange("n j h c -> n j (h c)"),
                    res8[:, 16 * half : 16 * half + 16, :],
                )
                nc.vector.tensor_scalar_max(fvs, fvs, 0.0)

            # high_int quarter: sum_n khi[n] * fv2 + bias_hi
            hi_sb = ap_.tile([1, QB, E], FP, tag="hi_sb", bufs=2)
            fv2f = fv2.rearrange("n b e -> n (b e)")
            for hf in range(2):
                pt = app.tile([1, 16, E], FP, tag="ps_q")
                ptf = pt.rearrange("o b e -> o (b e)")
                for j in range(4):
                    nc.tensor.matmul(
                        ptf[:, 512 * j : 512 * (j + 1)],
                        khi32h,
                        fv2f[:, 2048 * hf + 512 * j : 2048 * hf + 512 * (j + 1)],
                    )
                nc.vector.scalar_tensor_tensor(
                    out=hi_sb[:, 16 * hf : 16 * (hf + 1), :],
                    in0=pt,
                    scalar=1.0,
                    in1=_bc(bhi_sb, 1, 16),
                    op0=ALU.mult,
                    op1=ALU.add,
                )
            nc.sync.dma_start(out=out1_d[b0 : b0 + QB, 2 * C : 4 * C], in_=hi_sb)
    st_life.close()


_PROG_CACHE = {}


def _get_program(k_mf):
    key = float(k_mf)
    if key not in _PROG_CACHE:
        nc = bacc.Bacc("TRN2", target_bir_lowering=False, debug=False)
        with tile.TileContext(nc) as tc:
            with ExitStack() as ctx:
                emit_program(nc, tc, ctx, key)
        nc.compile()
        _PROG_CACHE[key] = nc
    return _PROG_CACHE[key]


def _prep_in_maps(inputs):
    f32 = lambda a: np.ascontiguousarray(np.asarray(a), dtype=np.float32)
    x = f32(inputs["x"])
    shared = {
        "w": f32(inputs["reweight_W"]),
        "rinit": f32(inputs["routing_init"]),
        "kfm": f32(inputs["kernel_fm"]).reshape(N),
        "bias_fm": f32(inputs["bias_fm"]).reshape(C),
        "bias_mf": f32(inputs["bias_mf"]).reshape(C),
        "khi": f32(inputs["kernel_highint"]).reshape(N),
        "bias_hi": f32(inputs["bias_highint"]).reshape(E),
        "wq": f32(inputs["W_query"]),
        "wk": f32(inputs["W_key"]),
        "wv": f32(inputs["W_value"]),
        "wres": f32(inputs["W_res"]),
    }
    in_maps = []
    for i in range(N_CORES):
        m = dict(shared)
        m["x"] = np.ascontiguousarray(x[i * BL : (i + 1) * BL])
        in_maps.append(m)
    k_mf = float(np.asarray(inputs["kernel_mf"]).reshape(-1)[0])
    return in_maps, k_mf


def kernel(**inputs):
    in_maps, k_mf = _prep_in_maps(inputs)
    nc = _get_program(k_mf)
    res = run_bass_kernel_spmd(nc, in_maps, list(range(N_CORES)))
    out1 = np.concatenate([res.results[i]["out1"] for i in range(N_CORES)], axis=0)
    rs = np.concatenate([res.results[i]["rs"] for i in range(N_CORES)], axis=0)
    return out1, rs.reshape(B, N, F, 1)
